# revision 12
# baseline (speedup 1.0000x reference)
"""FENet (7-layer stride-2 conv feature extractor) on 8 Trainium2 NeuronCores.

Strategy
--------
The whole network is linear except the |.| at each feature tap, so each of the
8 output features is  feat_f(b) = scale_f * sum_j |A_f @ x_b|_j  for a
host-precomputed composite banded matrix A_f (built in fp64 from the conv
weights, including all the inter-layer zero padding).  On device, per core:

  1. DMA 128-sample slabs of x (natural layout, contiguous).
  2. TensorE-transpose x into [position, sample] tiles (positions must sit on
     the partition/contraction axis for matmul).
  3. Banded matmul: the stacked A rows (1168 of them) are packed into 10
     row-blocks of <=128; per block only the k-chunks (128-column strips of
     the 900 input positions) where the block has support are multiplied --
     52 matmuls per 512-sample tile instead of 80 dense.
  4. ScalarE Abs from PSUM, then a per-block selector matmul (rows -> feature
     with the mean divisor folded in) accumulates all 8 features in PSUM.
  5. Tiny PE transpose of the [8, 512] feature tile back to [sample, 8] and
     a contiguous DMA out.

Data parallel over batch: 24576 samples -> 8 cores x 3072.
"""

import os
import sys

import numpy as np

for _p in ("/opt/trn_rl_repo", os.path.expanduser("~/.axon_site/_ro/trn_rl_repo")):
    if os.path.isdir(_p) and _p not in sys.path:
        sys.path.insert(0, _p)

import concourse.bass as bass
import concourse.bacc as bacc
import concourse.mybir as mybir
from concourse import tile
from concourse.bass_utils import run_bass_kernel_spmd

F32 = mybir.dt.float32
F32R = mybir.dt.float32r

N_CORES = 8
B_FULL = 24576
L_IN = 900
B_LOC = B_FULL // N_CORES          # 3072
N_TILE = 512                       # samples per matmul moving tile
N_GRP = 128                        # samples per transpose group
TILES = B_LOC // N_TILE            # 6
GRPS = N_TILE // N_GRP             # 4
NCH = 8                            # 900 = 7*128 + 4
CHW = [128] * 7 + [4]

KER, STR, PAD_L, PAD_R = 40, 2, 38, 39
N_LAYERS = 7
USE_F32R = False


# ----------------------------------------------------------------- host math
def _conv_map(M, w):
    """M: [L, 900] map from x to current positions; returns conv(pad(M)) map."""
    Mp = np.pad(M, ((PAD_L, PAD_R), (0, 0)))
    Lo = (Mp.shape[0] - KER) // STR + 1
    out = np.zeros((Lo, M.shape[1]), dtype=M.dtype)
    for k in range(KER):
        out += w[k] * Mp[k : k + STR * Lo : STR, :]
    return out


def _build_composite(feat_w, pass_w):
    """[(A_f [L_f, 900] fp64, scale_f)] for the 8 features."""
    P = np.eye(L_IN, dtype=np.float64)
    maps = []
    for i in range(N_LAYERS):
        F = _conv_map(P, feat_w[i, 0, 0].astype(np.float64))
        maps.append((F, 1.0 / F.shape[0]))
        P = _conv_map(P, pass_w[i, 0, 0].astype(np.float64))
    maps.append((P, 1.0 / 32.0))     # 2**round(log2(45)) == 32
    return maps


def _pack_blocks(maps):
    """Pack A rows into <=128-row blocks; narrow features 0/1 get their own
    block runs, the wide rest are concatenated.  Returns per-block row
    matrices, active k-chunks, and (feature, scale) per row."""
    rows = []
    for fid, (A, sc) in enumerate(maps):
        for r in range(A.shape[0]):
            rows.append((fid, sc, A[r]))
    n0 = maps[0][0].shape[0]
    n1 = maps[1][0].shape[0]
    groups = [rows[:n0], rows[n0 : n0 + n1], rows[n0 + n1 :]]
    blocks = []
    for g in groups:
        for s in range(0, len(g), 128):
            blk = g[s : s + 128]
            M = np.stack([v for _, _, v in blk])
            chs = [c for c in range(NCH)
                   if np.any(M[:, c * 128 : (c + 1) * 128] != 0.0)]
            blocks.append(dict(M=M, chunks=chs,
                               feats=[(f, sc) for f, sc, _ in blk]))
    return blocks


def _build_operands(blocks):
    """Device-side constant tensors: stacked lhsT tiles and selector tiles."""
    n_mm = sum(len(b["chunks"]) for b in blocks)
    n_blk = len(blocks)
    wt = np.zeros((n_mm, 128, 128), dtype=np.float32)
    sel = np.zeros((n_blk, 128, 8), dtype=np.float32)
    sched = []                        # per block: (mrows, [(mm_idx, chunk)...])
    i = 0
    for b, blk in enumerate(blocks):
        mrows = blk["M"].shape[0]
        ent = []
        for c in blk["chunks"]:
            kw = CHW[c]
            wt[i, :kw, :mrows] = blk["M"][:, c * 128 : c * 128 + kw].T
            ent.append((i, c))
            i += 1
        for r, (f, sc) in enumerate(blk["feats"]):
            sel[b, r, f] = sc
        sched.append((mrows, ent))
    # device SBUF layout: partition-major [k, i, m] / [r, b, f] so each loads
    # as one contiguous DMA
    wt = np.ascontiguousarray(wt.transpose(1, 0, 2))
    sel = np.ascontiguousarray(sel.transpose(1, 0, 2))
    return wt, sel, sched


# ------------------------------------------------------------ device program
def _build_program(sched, n_mm, n_blk):
    MMDT = F32R if USE_F32R else F32
    nc = bacc.Bacc()
    xs_d = nc.dram_tensor("xs", [B_LOC, L_IN], F32, kind="ExternalInput")
    wt_d = nc.dram_tensor("wt", [128, n_mm, 128], MMDT, kind="ExternalInput")
    sel_d = nc.dram_tensor("sel", [128, n_blk, 8], MMDT, kind="ExternalInput")
    id_d = nc.dram_tensor("ident", [128, 128], F32, kind="ExternalInput")
    out_d = nc.dram_tensor("out", [B_LOC, 8], F32, kind="ExternalOutput")

    with tile.TileContext(nc) as tc:
        with (
            tc.tile_pool(name="const", bufs=1) as constp,
            tc.tile_pool(name="xin", bufs=8) as xinp,
            tc.tile_pool(name="xt", bufs=2) as xtp,
            tc.tile_pool(name="za", bufs=3) as zap,
            tc.tile_pool(name="oute", bufs=2) as outp,
            tc.tile_pool(name="pt", bufs=3, space=bass.MemorySpace.PSUM) as ptp,
            tc.tile_pool(name="pz", bufs=2, space=bass.MemorySpace.PSUM) as pzp,
            tc.tile_pool(name="pf", bufs=2, space=bass.MemorySpace.PSUM) as pfp,
            tc.tile_pool(name="po", bufs=1, space=bass.MemorySpace.PSUM) as pop,
        ):
            wt_sb = constp.tile([128, n_mm, 128], MMDT)
            nc.gpsimd.dma_start(wt_sb[:], wt_d[:])
            sel_sb = constp.tile([128, n_blk, 8], MMDT)
            nc.gpsimd.dma_start(sel_sb[:], sel_d[:])
            id_sb = constp.tile([128, 128], F32)
            nc.gpsimd.dma_start(id_sb[:], id_d[:])

            for t in range(TILES):
                # ---- load + transpose 512 samples into [pos, sample] chunks
                xt_all = xtp.tile([128, NCH, N_TILE], MMDT, tag="xt")
                for g in range(GRPS):
                    row0 = (t * GRPS + g) * N_GRP
                    xn = xinp.tile([128, L_IN], F32, tag="xn")
                    nc.sync.dma_start(xn[:], xs_d[row0 : row0 + N_GRP, :])
                    for c in range(NCH):
                        cw = CHW[c]
                        pt = ptp.tile([cw, 128], F32, tag="pt")
                        nc.tensor.transpose(
                            pt[:], xn[:, c * 128 : c * 128 + cw], id_sb[:])
                        nc.vector.tensor_copy(
                            xt_all[0:cw, c, g * N_GRP : (g + 1) * N_GRP], pt[:])

                # ---- banded matmuls, abs, per-block feature reduce
                pf = pfp.tile([8, N_TILE], F32, tag="pf")
                for b, (mrows, ent) in enumerate(sched):
                    pz = pzp.tile([mrows, N_TILE], F32, tag="pz")
                    for j, (i, c) in enumerate(ent):
                        kw = CHW[c]
                        nc.tensor.matmul(
                            pz[:],
                            wt_sb[0:kw, i, 0:mrows],
                            xt_all[0:kw, c, :],
                            start=(j == 0), stop=(j == len(ent) - 1),
                            skip_group_check=True)
                    za = zap.tile([mrows, N_TILE], MMDT, tag="za")
                    nc.scalar.activation(
                        za[:], pz[:], mybir.ActivationFunctionType.Abs)
                    nc.tensor.matmul(
                        pf[:],
                        sel_sb[0:mrows, b, :],
                        za[:],
                        start=(b == 0), stop=(b == n_blk - 1),
                        skip_group_check=True)

                # ---- [8, 512] -> [512, 8] and store
                fc = outp.tile([8, N_TILE], F32, tag="fc")
                nc.vector.tensor_copy(fc[:], pf[:])
                for g in range(GRPS):
                    row0 = (t * GRPS + g) * N_GRP
                    po = pop.tile([128, 8], F32, tag="po")
                    nc.tensor.transpose(
                        po[:], fc[:, g * N_GRP : (g + 1) * N_GRP],
                        id_sb[0:8, 0:8])
                    ob = outp.tile([128, 8], F32, tag="ob")
                    nc.vector.tensor_copy(ob[:], po[:])
                    nc.sync.dma_start(out_d[row0 : row0 + N_GRP, :], ob[:])
    nc.finalize()
    return nc


_CACHE = {}


def _get_program(feat_w, pass_w):
    maps = _build_composite(feat_w, pass_w)
    blocks = _pack_blocks(maps)
    wt, sel, sched = _build_operands(blocks)
    key = tuple((m, tuple(e)) for m, e in sched)
    if key not in _CACHE:
        _CACHE[key] = _build_program(sched, wt.shape[1], sel.shape[1])
    return _CACHE[key], wt, sel


def kernel(x, feat_w, pass_w):
    nc, wt, sel = _get_program(feat_w, pass_w)
    ident = np.eye(128, dtype=np.float32)
    xs = np.ascontiguousarray(x.reshape(B_FULL, L_IN).astype(np.float32))
    in_maps = [
        {"xs": xs[i * B_LOC : (i + 1) * B_LOC],
         "wt": wt, "sel": sel, "ident": ident}
        for i in range(N_CORES)
    ]
    res = run_bass_kernel_spmd(nc, in_maps, list(range(N_CORES)))
    out = np.concatenate([res.results[i]["out"] for i in range(N_CORES)], axis=0)
    return np.ascontiguousarray(out.astype(np.float32))


# revision 13
# speedup vs baseline: 2.2717x; 2.2717x over previous
"""FENet (7-layer stride-2 conv feature extractor) on 8 Trainium2 NeuronCores.

Strategy
--------
The whole network is linear except the |.| at each feature tap, so each of the
8 output features is  feat_f(b) = scale_f * sum_j |A_f @ x_b|_j  for a
host-precomputed composite banded matrix A_f (built in fp64 from the conv
weights, including all the inter-layer zero padding).  On device, per core:

  1. DMA 128-sample slabs of x (natural layout, contiguous).
  2. TensorE-transpose x into [position, sample] tiles (positions must sit on
     the partition/contraction axis for matmul).
  3. Banded matmul: the stacked A rows (1168 of them) are packed into 10
     row-blocks of <=128; per block only the k-chunks (128-column strips of
     the 900 input positions) where the block has support are multiplied --
     52 matmuls per 512-sample tile instead of 80 dense.
  4. ScalarE Abs from PSUM, then a per-block selector matmul (rows -> feature
     with the mean divisor folded in) accumulates all 8 features in PSUM.
  5. Tiny PE transpose of the [8, 512] feature tile back to [sample, 8] and
     a contiguous DMA out.

Data parallel over batch: 24576 samples -> 8 cores x 3072.
"""

import os
import sys

import numpy as np

for _p in ("/opt/trn_rl_repo", os.path.expanduser("~/.axon_site/_ro/trn_rl_repo")):
    if os.path.isdir(_p) and _p not in sys.path:
        sys.path.insert(0, _p)

import concourse.bass as bass
import concourse.bacc as bacc
import concourse.mybir as mybir
from concourse import tile
from concourse.bass_utils import run_bass_kernel_spmd

F32 = mybir.dt.float32
F32R = mybir.dt.float32r

N_CORES = 8
B_FULL = 24576
L_IN = 900
B_LOC = B_FULL // N_CORES          # 3072
N_TILE = 512                       # samples per matmul moving tile
N_GRP = 128                        # samples per transpose group
TILES = B_LOC // N_TILE            # 6
GRPS = N_TILE // N_GRP             # 4
NCH = 8                            # 900 = 7*128 + 4
CHW = [128] * 7 + [4]

KER, STR, PAD_L, PAD_R = 40, 2, 38, 39
N_LAYERS = 7
USE_F32R = True


# ----------------------------------------------------------------- host math
def _conv_map(M, w):
    """M: [L, 900] map from x to current positions; returns conv(pad(M)) map."""
    Mp = np.pad(M, ((PAD_L, PAD_R), (0, 0)))
    Lo = (Mp.shape[0] - KER) // STR + 1
    out = np.zeros((Lo, M.shape[1]), dtype=M.dtype)
    for k in range(KER):
        out += w[k] * Mp[k : k + STR * Lo : STR, :]
    return out


def _build_composite(feat_w, pass_w):
    """[(A_f [L_f, 900] fp64, scale_f)] for the 8 features."""
    P = np.eye(L_IN, dtype=np.float64)
    maps = []
    for i in range(N_LAYERS):
        F = _conv_map(P, feat_w[i, 0, 0].astype(np.float64))
        maps.append((F, 1.0 / F.shape[0]))
        P = _conv_map(P, pass_w[i, 0, 0].astype(np.float64))
    maps.append((P, 1.0 / 32.0))     # 2**round(log2(45)) == 32
    return maps


def _pack_blocks(maps):
    """Pack A rows into <=128-row blocks; narrow features 0/1 get their own
    block runs, the wide rest are concatenated.  Returns per-block row
    matrices, active k-chunks, and (feature, scale) per row."""
    rows = []
    for fid, (A, sc) in enumerate(maps):
        for r in range(A.shape[0]):
            rows.append((fid, sc, A[r]))
    n0 = maps[0][0].shape[0]
    n1 = maps[1][0].shape[0]
    groups = [rows[:n0], rows[n0 : n0 + n1], rows[n0 + n1 :]]
    blocks = []
    for g in groups:
        for s in range(0, len(g), 128):
            blk = g[s : s + 128]
            M = np.stack([v for _, _, v in blk])
            chs = [c for c in range(NCH)
                   if np.any(M[:, c * 128 : (c + 1) * 128] != 0.0)]
            blocks.append(dict(M=M, chunks=chs,
                               feats=[(f, sc) for f, sc, _ in blk]))
    return blocks


def _build_operands(blocks):
    """Device-side constant tensors: stacked lhsT tiles and selector tiles."""
    n_mm = sum(len(b["chunks"]) for b in blocks)
    n_blk = len(blocks)
    wt = np.zeros((n_mm, 128, 128), dtype=np.float32)
    sel = np.zeros((n_blk, 128, 8), dtype=np.float32)
    sched = []                        # per block: (mrows, [(mm_idx, chunk)...])
    i = 0
    for b, blk in enumerate(blocks):
        mrows = blk["M"].shape[0]
        ent = []
        for c in blk["chunks"]:
            kw = CHW[c]
            wt[i, :kw, :mrows] = blk["M"][:, c * 128 : c * 128 + kw].T
            ent.append((i, c))
            i += 1
        for r, (f, sc) in enumerate(blk["feats"]):
            sel[b, r, f] = sc
        sched.append((mrows, ent))
    # device SBUF layout: partition-major [k, i, m] / [r, b, f] so each loads
    # as one contiguous DMA
    wt = np.ascontiguousarray(wt.transpose(1, 0, 2))
    sel = np.ascontiguousarray(sel.transpose(1, 0, 2))
    return wt, sel, sched


# ------------------------------------------------------------ device program
def _build_program(sched, n_mm, n_blk):
    MMDT = F32R if USE_F32R else F32
    nc = bacc.Bacc()
    xs_d = nc.dram_tensor("xs", [B_LOC, L_IN], F32, kind="ExternalInput")
    wt_d = nc.dram_tensor("wt", [128, n_mm, 128], MMDT, kind="ExternalInput")
    sel_d = nc.dram_tensor("sel", [128, n_blk, 8], MMDT, kind="ExternalInput")
    id_d = nc.dram_tensor("ident", [128, 128], F32, kind="ExternalInput")
    out_d = nc.dram_tensor("out", [B_LOC, 8], F32, kind="ExternalOutput")

    with tile.TileContext(nc) as tc:
        with (
            tc.tile_pool(name="const", bufs=1) as constp,
            tc.tile_pool(name="xin", bufs=8) as xinp,
            tc.tile_pool(name="xt", bufs=2) as xtp,
            tc.tile_pool(name="za", bufs=3) as zap,
            tc.tile_pool(name="oute", bufs=2) as outp,
            tc.tile_pool(name="pt", bufs=3, space=bass.MemorySpace.PSUM) as ptp,
            tc.tile_pool(name="pz", bufs=2, space=bass.MemorySpace.PSUM) as pzp,
            tc.tile_pool(name="pf", bufs=2, space=bass.MemorySpace.PSUM) as pfp,
            tc.tile_pool(name="po", bufs=1, space=bass.MemorySpace.PSUM) as pop,
        ):
            wt_sb = constp.tile([128, n_mm, 128], MMDT)
            nc.gpsimd.dma_start(wt_sb[:], wt_d[:])
            sel_sb = constp.tile([128, n_blk, 8], MMDT)
            nc.gpsimd.dma_start(sel_sb[:], sel_d[:])
            id_sb = constp.tile([128, 128], F32)
            nc.gpsimd.dma_start(id_sb[:], id_d[:])

            for t in range(TILES):
                # ---- load + transpose 512 samples into [pos, sample] chunks
                xt_all = xtp.tile([128, NCH, N_TILE], MMDT, tag="xt")
                for g in range(GRPS):
                    row0 = (t * GRPS + g) * N_GRP
                    xn = xinp.tile([128, L_IN], F32, tag="xn")
                    nc.sync.dma_start(xn[:], xs_d[row0 : row0 + N_GRP, :])
                    for c in range(NCH):
                        cw = CHW[c]
                        pt = ptp.tile([cw, 128], F32, tag="pt")
                        nc.tensor.transpose(
                            pt[:], xn[:, c * 128 : c * 128 + cw], id_sb[:])
                        nc.vector.tensor_copy(
                            xt_all[0:cw, c, g * N_GRP : (g + 1) * N_GRP], pt[:])

                # ---- banded matmuls, abs, per-block feature reduce
                pf = pfp.tile([8, N_TILE], F32, tag="pf")
                for b, (mrows, ent) in enumerate(sched):
                    pz = pzp.tile([mrows, N_TILE], F32, tag="pz")
                    for j, (i, c) in enumerate(ent):
                        kw = CHW[c]
                        nc.tensor.matmul(
                            pz[:],
                            wt_sb[0:kw, i, 0:mrows],
                            xt_all[0:kw, c, :],
                            start=(j == 0), stop=(j == len(ent) - 1),
                            skip_group_check=True)
                    za = zap.tile([mrows, N_TILE], MMDT, tag="za")
                    nc.scalar.activation(
                        za[:], pz[:], mybir.ActivationFunctionType.Abs)
                    nc.tensor.matmul(
                        pf[:],
                        sel_sb[0:mrows, b, :],
                        za[:],
                        start=(b == 0), stop=(b == n_blk - 1),
                        skip_group_check=True)

                # ---- [8, 512] -> [512, 8] and store
                fc = outp.tile([8, N_TILE], F32, tag="fc")
                nc.vector.tensor_copy(fc[:], pf[:])
                for g in range(GRPS):
                    row0 = (t * GRPS + g) * N_GRP
                    po = pop.tile([128, 8], F32, tag="po")
                    nc.tensor.transpose(
                        po[:], fc[:, g * N_GRP : (g + 1) * N_GRP],
                        id_sb[0:8, 0:8])
                    ob = outp.tile([128, 8], F32, tag="ob")
                    nc.vector.tensor_copy(ob[:], po[:])
                    nc.sync.dma_start(out_d[row0 : row0 + N_GRP, :], ob[:])
    nc.finalize()
    return nc


_CACHE = {}


def _get_program(feat_w, pass_w):
    maps = _build_composite(feat_w, pass_w)
    blocks = _pack_blocks(maps)
    wt, sel, sched = _build_operands(blocks)
    key = tuple((m, tuple(e)) for m, e in sched)
    if key not in _CACHE:
        _CACHE[key] = _build_program(sched, wt.shape[1], sel.shape[1])
    return _CACHE[key], wt, sel


def kernel(x, feat_w, pass_w):
    nc, wt, sel = _get_program(feat_w, pass_w)
    ident = np.eye(128, dtype=np.float32)
    xs = np.ascontiguousarray(x.reshape(B_FULL, L_IN).astype(np.float32))
    in_maps = [
        {"xs": xs[i * B_LOC : (i + 1) * B_LOC],
         "wt": wt, "sel": sel, "ident": ident}
        for i in range(N_CORES)
    ]
    res = run_bass_kernel_spmd(nc, in_maps, list(range(N_CORES)))
    out = np.concatenate([res.results[i]["out"] for i in range(N_CORES)], axis=0)
    return np.ascontiguousarray(out.astype(np.float32))
